# revision 5
# baseline (speedup 1.0000x reference)
"""MLA attention kernel for TRN2, SPMD over 8 NeuronCores.

Sharding: core c = 4*b + g  (b = batch 0..1, g = head-group 0..3, 4 heads each).
Each core computes, for its batch b and head-group g:
    qT = (Wq_g*scale)^T x^T + bq_g*scale        [256, 2048]   (bf16)
    latT = Wl^T x^T + bl                        [256, 2048]
    kT = Wk_g^T latT                            [256, 2048]   (bk dropped: softmax shift-invariant)
    v  = latT^T Wv_g                            [2048, 256]   (bv folded into host const)
    per head h: sT = kT_h^T qT_h ; pT = exp(sT) (no max-subtraction; scores ~ N(0,1))
                PV with [V_h | 1] augmented weights: psum rows 0:64 = O_h, row 64 = L_h
                at = O_h / L_h  (recip_approx_fast + PE ones-broadcast + DVE mult)
    partial = A Wo_g                            [2048, 1024]  (f32)
Host sums the 4 partials per batch and adds (bv @ Wo + bo).
"""
import contextlib
import ctypes
import os
import sys
import types

if "/opt/trn_rl_repo" not in sys.path:
    sys.path.insert(0, "/opt/trn_rl_repo")

import numpy as np
import ml_dtypes

NPBF16 = ml_dtypes.bfloat16
SCALE = 64 ** -0.5
_STATE = {}


# ---------------------------------------------------------------- ntff shim
def _install_ntff_shim():
    """Provide antenv.axon_hooks so run_bass_kernel_spmd(trace=True) works."""
    if "antenv.axon_hooks" in sys.modules:
        return
    try:
        import antenv
    except ImportError:
        return

    so_path = "/opt/axon/libaxon_pjrt.so"

    def _hook_factory():
        try:
            lib = ctypes.CDLL(so_path)
        except OSError:
            return None
        if not hasattr(lib, "axon_start_nrt_profile"):
            return None
        lib.axon_start_nrt_profile.argtypes = [ctypes.POINTER(ctypes.c_int64), ctypes.c_size_t]
        lib.axon_start_nrt_profile.restype = ctypes.c_int64
        lib.axon_stop_nrt_profile.argtypes = [ctypes.c_char_p]
        lib.axon_stop_nrt_profile.restype = ctypes.c_int64

        @contextlib.contextmanager
        def _hook(output_dir, device_ids):
            import jax

            jax.devices()
            if device_ids:
                ids = (ctypes.c_int64 * len(device_ids))(*device_ids)
                rc = lib.axon_start_nrt_profile(ids, len(device_ids))
            else:
                rc = lib.axon_start_nrt_profile(None, 0)
            if rc != 0:
                raise RuntimeError(f"axon_start_nrt_profile rc={rc}")
            try:
                yield
            finally:
                n = lib.axon_stop_nrt_profile(str(output_dir).encode())
                print(f"profile: {n} file(s) written to {output_dir}", file=sys.stderr)

        return _hook

    import antenv

    mod = types.ModuleType("antenv.axon_hooks")
    _state = {"hook": _hook_factory()}
    mod.set_axon_ntff_profile_hook = lambda h: _state.__setitem__("hook", h)
    mod.get_axon_ntff_profile_hook = lambda: _state["hook"]
    sys.modules["antenv.axon_hooks"] = mod
    antenv.axon_hooks = mod


# ---------------------------------------------------------------- bass build
def _build_nc(debug_dump=False):
    import concourse.bass as bass  # noqa: F401
    import concourse.tile as tile
    from concourse import bacc, mybir

    BF16 = mybir.dt.bfloat16
    F32 = mybir.dt.float32
    EXP = mybir.ActivationFunctionType.Exp
    CPY = mybir.ActivationFunctionType.Copy
    LN = mybir.ActivationFunctionType.Ln

    nc = bacc.Bacc(None, target_bir_lowering=False, debug=False)

    xT = nc.dram_tensor("xT", [128, 8, 2048], BF16, kind="ExternalInput")
    wq = nc.dram_tensor("wq", [128, 8, 256], BF16, kind="ExternalInput")
    bq = nc.dram_tensor("bq", [128, 2], F32, kind="ExternalInput")
    wl = nc.dram_tensor("wl", [128, 8, 256], BF16, kind="ExternalInput")
    bl = nc.dram_tensor("bl", [128, 2], F32, kind="ExternalInput")
    wk = nc.dram_tensor("wk", [128, 2, 256], BF16, kind="ExternalInput")
    wv = nc.dram_tensor("wv", [128, 2, 256], BF16, kind="ExternalInput")
    wo = nc.dram_tensor("wo", [128, 2, 1024], BF16, kind="ExternalInput")
    out = nc.dram_tensor("out", [2048, 1024], F32, kind="ExternalOutput")

    with nc.allow_low_precision("bf16 intermediates by design"), tile.TileContext(nc) as tc:
        with (
            tc.tile_pool(name="wpool", bufs=1) as wpool,
            tc.tile_pool(name="xpool", bufs=1) as xpool,
            tc.tile_pool(name="proj", bufs=1) as proj,
            tc.tile_pool(name="ptp", bufs=36) as ptp,
            tc.tile_pool(name="atp", bufs=4) as atp,
            tc.tile_pool(name="obp", bufs=4) as obp,
            tc.tile_pool(name="rpool", bufs=2) as rpool,
            tc.tile_pool(name="ps", bufs=2, space="PSUM") as ps,
        ):
            # ---------------- constants + inputs
            x_kn = [
                [xpool.tile([128, 512], BF16, name=f"x_{k}_{n}") for n in range(4)]
                for k in range(8)
            ]
            wq_sb = wpool.tile([128, 8, 256], BF16)
            wl_sb = wpool.tile([128, 8, 256], BF16)
            wk_sb = wpool.tile([128, 2, 256], BF16)
            wv_sb = wpool.tile([128, 2, 256], BF16)
            wo_sb = wpool.tile([128, 2, 1024], BF16)
            bq_sb = wpool.tile([128, 2], F32)
            bl_sb = wpool.tile([128, 2], F32)
            ones_k1 = wpool.tile([128, 64], BF16)
            nc.vector.memset(ones_k1[:], 1.0)

            nc.sync.dma_start(out=wq_sb[:], in_=wq[:])
            nc.sync.dma_start(out=bq_sb[:], in_=bq[:])
            nc.sync.dma_start(out=wl_sb[:], in_=wl[:])
            nc.sync.dma_start(out=bl_sb[:], in_=bl[:])
            nc.sync.dma_start(out=wk_sb[:], in_=wk[:])
            for n in range(4):
                for k in range(8):
                    nc.sync.dma_start(
                        out=x_kn[k][n][:],
                        in_=xT[:, k, 512 * n : 512 * n + 512],
                    )
            nc.sync.dma_start(out=wv_sb[:], in_=wv[:])
            nc.sync.dma_start(out=wo_sb[:], in_=wo[:])

            latT_n = [proj.tile([128, 2, 512], BF16, name=f"latT_{i}") for i in range(4)]
            qT_n = [proj.tile([128, 2, 512], BF16, name=f"qT_{i}") for i in range(4)]
            kT_n = [proj.tile([128, 2, 512], BF16, name=f"kT_{i}") for i in range(4)]
            # V with ones column: v2[:, t, h, 0:64] = V_h chunk t, v2[:, t, h, 64] = 1
            v2 = proj.tile([128, 16, 4, 65], BF16)
            nc.vector.memset(v2[:, :, :, 64:65], 1.0)

            def misc_ps(name, shape=(128, 512)):
                return ps.tile(list(shape), F32, tag="m", name=name, bufs=2)

            # HAM warm-up: dummy matmuls while input DMA is in flight
            warm_sb = wpool.tile([128, 512], BF16)
            nc.vector.memset(warm_sb[:], 0.25)
            warm_ps = ps.tile([128, 512], F32, tag="m", name="warm_ps", bufs=2)
            for i in range(40):
                nc.tensor.matmul(
                    warm_ps[:], warm_sb[:, 0:128], warm_sb[:],
                    start=(i == 0), stop=(i == 39),
                )

            # ---------------- projection emitters (interleaved as fillers)
            def emit_lat(n, m):
                acc = misc_ps(f"lat_ps_{m}_{n}")
                for k in range(8):
                    nc.tensor.matmul(
                        acc[:],
                        wl_sb[:, k, 128 * m : 128 * m + 128],
                        x_kn[k][n][:],
                        start=(k == 0),
                        stop=(k == 7),
                    )
                nc.vector.tensor_scalar_add(
                    out=latT_n[n][:, m, :], in0=acc[:], scalar1=bl_sb[:, m : m + 1]
                )

            def emit_kt(n):
                for m in range(2):
                    acc = misc_ps(f"kt_ps_{m}_{n}")
                    for k in range(2):
                        nc.tensor.matmul(
                            acc[:],
                            wk_sb[:, k, 128 * m : 128 * m + 128],
                            latT_n[n][:, k, :],
                            start=(k == 0),
                            stop=(k == 1),
                        )
                    nc.vector.tensor_copy(out=kT_n[n][:, m, :], in_=acc[:])

            def emit_v(ts):
                for t in ts:
                    acc = misc_ps(f"v_ps_{t}", shape=(128, 4, 64))
                    for k in range(2):
                        nc.tensor.matmul(
                            acc[:, :, :],
                            latT_n[t // 4][:, k, 128 * (t % 4) : 128 * (t % 4) + 128],
                            wv_sb[:, k, :],
                            start=(k == 0),
                            stop=(k == 1),
                        )
                    nc.vector.tensor_copy(out=v2[:, t, :, 0:64], in_=acc[:, :, :])

            def emit_qt(n, m):
                acc = misc_ps(f"q_ps_{m}_{n}")
                for k in range(8):
                    nc.tensor.matmul(
                        acc[:],
                        wq_sb[:, k, 128 * m : 128 * m + 128],
                        x_kn[k][n][:],
                        start=(k == 0),
                        stop=(k == 7),
                    )
                nc.vector.tensor_scalar_add(
                    out=qT_n[n][:, m, :], in0=acc[:], scalar1=bq_sb[:, m : m + 1]
                )

            # ---------------- attention phase machinery
            # state per pair key (ic, p): {"pt": [16 tiles], "otA","otB", "Lsb","rec16","uJ","at"}
            PD = {}

            def emit_phase(ic, p, fillers):
                """QK + exp for pair (ic,p), kchunk-granular, fillers[t] = list of closures."""
                key = (ic, p)
                PD[key] = {"pt": []}
                qTc = qT_n[ic]
                for t in range(16):
                    kTc = kT_n[t // 4]
                    ksl = slice(128 * (t % 4), 128 * (t % 4) + 128)
                    s = ps.tile([128, 2, 512], F32, tag="s", name=f"s_{ic}_{p}_{t}", bufs=2)
                    nc.tensor.matmul(
                        s[:, 0, :], kTc[0:64, p, ksl], qTc[0:64, p, :],
                        start=True, stop=True,
                    )
                    nc.tensor.matmul(
                        s[:, 1, :], kTc[64:128, p, ksl], qTc[64:128, p, :],
                        start=True, stop=True,
                    )
                    pt = ptp.tile([128, 2, 512], BF16, tag="pt", name=f"pt_{ic}_{p}_{t}")
                    nc.scalar.activation(pt[:], s[:], EXP)
                    PD[key]["pt"].append(pt)
                    for f in fillers.get(t, ()):
                        f()

            def emit_pv(key, t):
                """One PV kchunk for pair key; accumulates [O_h; L_h] in 65-row psums."""
                d = PD[key]
                ic, p = key
                if t == 0:
                    d["otA"] = ps.tile([128, 512], F32, tag="ot", name=f"otA_{ic}_{p}", bufs=2)
                    d["otB"] = ps.tile([128, 512], F32, tag="ot", name=f"otB_{ic}_{p}", bufs=2)
                pt = d["pt"][t]
                nc.tensor.matmul(
                    d["otA"][0:65, :], v2[:, t, 2 * p, :], pt[:, 0, :],
                    start=(t == 0), stop=(t == 15), skip_group_check=True,
                )
                nc.tensor.matmul(
                    d["otB"][0:65, :], v2[:, t, 2 * p + 1, :], pt[:, 1, :],
                    start=(t == 0), stop=(t == 15), skip_group_check=True,
                )

            def emit_normA(key):
                """After PV stop: 1/L = exp(-ln L) on Act (L sits at psum row 64)."""
                d = PD[key]
                ic, p = key
                rec16s = []
                for j, ot in ((0, d["otA"]), (1, d["otB"])):
                    lnL = rpool.tile([65, 512], F32, tag="lnL", name=f"lnL_{ic}_{p}_{j}", bufs=4)
                    nc.scalar.activation(lnL[64:65, :], ot[64:65, :], LN)
                    rec16 = rpool.tile([65, 512], BF16, tag="rec16", name=f"rec16_{ic}_{p}_{j}", bufs=4)
                    nc.scalar.activation(rec16[64:65, :], lnL[64:65, :], EXP, scale=-1.0)
                    rec16s.append(rec16)
                d["rec16"] = rec16s
                # unnormalized O copy (Act) frees the ot psum banks early
                uJ = atp.tile([64, 2, 512], BF16, tag="uJ", name=f"uJ_{ic}_{p}", bufs=2)
                nc.scalar.activation(uJ[:, 0, :], d["otA"][0:64, :], CPY)
                nc.scalar.activation(uJ[:, 1, :], d["otB"][0:64, :], CPY)
                d["uJ"] = uJ

            def emit_normB(key):
                """PE ones-broadcast of recip, DVE mult, DMA-assemble at[128,512]."""
                d = PD[key]
                ic, p = key
                bcs = []
                for j in range(2):
                    bc_ps = misc_ps(f"bcp_{ic}_{p}_{j}", shape=(64, 512))
                    nc.tensor.matmul(
                        bc_ps[0:64, :],
                        ones_k1[64:65, 0:64],
                        d["rec16"][j][64:65, :],
                        start=True, stop=True,
                        tile_position=(64, 0),
                        skip_group_check=True,
                    )
                    bc = rpool.tile([64, 512], BF16, tag="bc", name=f"bcs_{ic}_{p}_{j}", bufs=4)
                    nc.vector.tensor_copy(out=bc[:], in_=bc_ps[0:64, :])
                    bcs.append(bc)
                atJ = atp.tile([64, 2, 512], BF16, tag="atJ", name=f"atJ_{ic}_{p}", bufs=2)
                for j in range(2):
                    nc.vector.tensor_mul(
                        out=atJ[:, j, :], in0=d["uJ"][:, j, :], in1=bcs[j][:]
                    )
                at = atp.tile([128, 512], BF16, tag="at", name=f"at_{ic}_{p}", bufs=4)
                nc.sync.dma_start(out=at[0:64, :], in_=atJ[:, 0, :])
                nc.sync.dma_start(out=at[64:128, :], in_=atJ[:, 1, :])
                d["at"] = at

            def emit_wo_chunk(ic, u):
                at0 = PD[(ic, 0)]["at"]
                at1 = PD[(ic, 1)]["at"]
                for n2 in range(2):
                    wo_ps = misc_ps(f"wo_{ic}_{u}_{n2}")
                    for p, atx in ((0, at0), (1, at1)):
                        nc.tensor.matmul(
                            wo_ps[:],
                            atx[:, 128 * u : 128 * u + 128],
                            wo_sb[:, p, 512 * n2 : 512 * n2 + 512],
                            start=(p == 0),
                            stop=(p == 1),
                        )
                    ob = obp.tile([128, 512], F32, tag="ob", name=f"ob_{ic}_{u}_{n2}")
                    nc.vector.tensor_copy(out=ob[:], in_=wo_ps[:])
                    r0 = 512 * ic + 128 * u
                    nc.sync.dma_start(
                        out=out[r0 : r0 + 128, 512 * n2 : 512 * n2 + 512],
                        in_=ob[:],
                    )

            # ---------------- schedule
            # prologue
            emit_qt(0, 0)
            emit_qt(0, 1)
            emit_lat(0, 0)
            emit_lat(0, 1)
            emit_kt(0)

            PAIRS = [(ic, p) for ic in range(4) for p in range(2)]

            def pv_fillers(prev_key, extra=None):
                """Spread 16 pv chunks of prev pair over slots t2..t15; extra[t] prepended."""
                f = {t: [] for t in range(16)}
                if extra:
                    for t, fns in extra.items():
                        f[t].extend(fns)
                slots = [2, 2, 3, 4, 5, 6, 7, 8, 9, 10, 11, 12, 12, 13, 14, 15]
                for k, t in enumerate(slots):
                    f[t].append(lambda k=k: emit_pv(prev_key, k))
                f[15].append(lambda: emit_normA(prev_key))
                return f

            # P0: pair (0,0) — projections as fillers
            emit_phase(0, 0, {
                0: [lambda: emit_lat(1, 0)],
                1: [lambda: emit_lat(1, 1)],
                2: [lambda: emit_kt(1), lambda: emit_v(range(0, 2))],
                3: [lambda: emit_v(range(2, 4))],
                4: [lambda: emit_lat(2, 0)],
                5: [lambda: emit_lat(2, 1)],
                6: [lambda: emit_kt(2), lambda: emit_v(range(4, 6))],
                7: [lambda: emit_v(range(6, 8))],
                8: [lambda: emit_lat(3, 0)],
                9: [lambda: emit_lat(3, 1)],
                10: [lambda: emit_kt(3), lambda: emit_v(range(8, 10))],
                11: [lambda: emit_v(range(10, 12))],
                12: [lambda: emit_v(range(12, 14))],
                13: [lambda: emit_v(range(14, 16))],
                14: [lambda: emit_qt(1, 0)],
                15: [lambda: emit_qt(1, 1)],
            })
            # P1: pair (0,1) — qt(2) + PV(P0)
            emit_phase(0, 1, pv_fillers((0, 0), extra={
                0: [lambda: emit_qt(2, 0)],
                1: [lambda: emit_qt(2, 1)],
            }))
            # P2: pair (1,0) — normB(P0) + qt(3) + PV(P1)
            emit_phase(1, 0, pv_fillers((0, 1), extra={
                0: [lambda: emit_normB((0, 0)), lambda: emit_qt(3, 0)],
                1: [lambda: emit_qt(3, 1)],
            }))
            # P3: pair (1,1) — normB(P1) + Wo(0) + PV(P2)
            emit_phase(1, 1, pv_fillers((1, 0), extra={
                0: [lambda: emit_normB((0, 1))],
                **{4 + u: [lambda u=u: emit_wo_chunk(0, u)] for u in range(4)},
            }))
            # P4: pair (2,0) — normB(P2) + PV(P3)
            emit_phase(2, 0, pv_fillers((1, 1), extra={
                0: [lambda: emit_normB((1, 0))],
            }))
            # P5: pair (2,1) — normB(P3) + Wo(1) + PV(P4)
            emit_phase(2, 1, pv_fillers((2, 0), extra={
                0: [lambda: emit_normB((1, 1))],
                **{4 + u: [lambda u=u: emit_wo_chunk(1, u)] for u in range(4)},
            }))
            # P6: pair (3,0) — normB(P4) + PV(P5)
            emit_phase(3, 0, pv_fillers((2, 1), extra={
                0: [lambda: emit_normB((2, 0))],
            }))
            # P7: pair (3,1) — normB(P5) + Wo(2) + PV(P6)
            emit_phase(3, 1, pv_fillers((3, 0), extra={
                0: [lambda: emit_normB((2, 1))],
                **{4 + u: [lambda u=u: emit_wo_chunk(2, u)] for u in range(4)},
            }))
            # tail
            emit_normB((3, 0))
            for k in range(16):
                emit_pv((3, 1), k)
            emit_normA((3, 1))
            emit_normB((3, 1))
            for u in range(4):
                emit_wo_chunk(3, u)

    nc.compile()
    return nc


def _get_nc():
    if "nc" not in _STATE:
        _STATE["nc"] = _build_nc()
    return _STATE["nc"]


# ---------------------------------------------------------------- host side
def _pack_k(a, kchunks):
    """[K, N] f32/bf16 -> [128, kchunks, N] bf16 (K = 128*kchunks)."""
    K, N = a.shape
    return np.ascontiguousarray(
        np.asarray(a, np.float32).reshape(kchunks, 128, N).transpose(1, 0, 2)
    ).astype(NPBF16)


def kernel(x, Wq, bq, Wl, bl, Wk, bk, Wv, bv, Wo, bo):
    x = np.asarray(x, np.float32)
    Wq = np.asarray(Wq, np.float32)
    bq = np.asarray(bq, np.float32)
    Wl = np.asarray(Wl, np.float32)
    bl = np.asarray(bl, np.float32)
    Wk = np.asarray(Wk, np.float32)
    Wv = np.asarray(Wv, np.float32)
    bv = np.asarray(bv, np.float32)
    Wo = np.asarray(Wo, np.float32)
    bo = np.asarray(bo, np.float32)

    from concourse.bass_utils import run_bass_kernel_spmd

    trace = os.environ.get("KERNEL_TRACE", "0") == "1"
    if trace:
        _install_ntff_shim()

    wl_p = _pack_k(Wl, 8)
    bl_p = np.ascontiguousarray(bl.reshape(2, 128).T).astype(np.float32)
    in_maps = []
    for c in range(8):
        b, g = divmod(c, 4)
        sl = slice(256 * g, 256 * g + 256)
        in_maps.append(
            {
                "xT": _pack_k(x[b].T, 8),
                "wq": _pack_k(Wq[:, sl] * SCALE, 8),
                "bq": np.ascontiguousarray((bq[sl] * SCALE).reshape(2, 128).T).astype(np.float32),
                "wl": wl_p,
                "bl": bl_p,
                "wk": _pack_k(Wk[:, sl], 2),
                "wv": _pack_k(Wv[:, sl], 2),
                "wo": _pack_k(Wo[sl, :], 2),
            }
        )

    nc = _get_nc()
    res = run_bass_kernel_spmd(nc, in_maps, core_ids=list(range(8)), trace=trace)
    if trace and res.exec_time_ns is not None:
        print(f"HW exec time: {res.exec_time_ns} ns")
        _STATE["exec_time_ns"] = res.exec_time_ns

    parts = [np.asarray(res.results[c]["out"], np.float32) for c in range(8)]
    const = (bv @ Wo + bo).astype(np.float32)
    out = np.empty((2, 2048, 1024), np.float32)
    for b in range(2):
        out[b] = parts[4 * b] + parts[4 * b + 1] + parts[4 * b + 2] + parts[4 * b + 3] + const
    return out


# revision 14
# speedup vs baseline: 1.0238x; 1.0238x over previous
"""MLA attention kernel for TRN2, SPMD over 8 NeuronCores.

Sharding: core c = 4*b + g  (b = batch 0..1, g = head-group 0..3, 4 heads each).
Each core computes, for its batch b and head-group g:
    qT = (Wq_g*scale)^T x^T + bq_g*scale        [256, 2048]   (bf16)
    latT = Wl^T x^T + bl                        [256, 2048]
    kT = Wk_g^T latT                            [256, 2048]   (bk dropped: softmax shift-invariant)
    v  = latT^T Wv_g                            [2048, 256]   (bv folded into host const)
    per head h: sT = kT_h^T qT_h ; pT = exp(sT) (no max-subtraction; scores ~ N(0,1))
                oT = v_h^T pT ; L = 1^T pT ; aT = oT / L
    partial = A Wo_g                            [2048, 1024]  (f32)
Host sums the 4 partials per batch and adds (bv @ Wo + bo).

Emission is kchunk-granular: per k-chunk t the two QK matmuls (row-groups 0/64)
are adjacent for PE row-tile concurrency; PV/L matmuls are ordered
[PVh0, L_h1, PVh1, L_h0] with disjoint col-groups for the same reason.
"""
import contextlib
import ctypes
import os
import sys
import types

if "/opt/trn_rl_repo" not in sys.path:
    sys.path.insert(0, "/opt/trn_rl_repo")

import numpy as np
import ml_dtypes

NPBF16 = ml_dtypes.bfloat16
SCALE = 64 ** -0.5
_STATE = {}


# ---------------------------------------------------------------- ntff shim
def _install_ntff_shim():
    """Provide antenv.axon_hooks so run_bass_kernel_spmd(trace=True) works."""
    if "antenv.axon_hooks" in sys.modules:
        return
    try:
        import antenv
    except ImportError:
        return

    so_path = "/opt/axon/libaxon_pjrt.so"

    def _hook_factory():
        try:
            lib = ctypes.CDLL(so_path)
        except OSError:
            return None
        if not hasattr(lib, "axon_start_nrt_profile"):
            return None
        lib.axon_start_nrt_profile.argtypes = [ctypes.POINTER(ctypes.c_int64), ctypes.c_size_t]
        lib.axon_start_nrt_profile.restype = ctypes.c_int64
        lib.axon_stop_nrt_profile.argtypes = [ctypes.c_char_p]
        lib.axon_stop_nrt_profile.restype = ctypes.c_int64

        @contextlib.contextmanager
        def _hook(output_dir, device_ids):
            import jax

            jax.devices()
            if device_ids:
                ids = (ctypes.c_int64 * len(device_ids))(*device_ids)
                rc = lib.axon_start_nrt_profile(ids, len(device_ids))
            else:
                rc = lib.axon_start_nrt_profile(None, 0)
            if rc != 0:
                raise RuntimeError(f"axon_start_nrt_profile rc={rc}")
            try:
                yield
            finally:
                n = lib.axon_stop_nrt_profile(str(output_dir).encode())
                print(f"profile: {n} file(s) written to {output_dir}", file=sys.stderr)

        return _hook

    import antenv

    mod = types.ModuleType("antenv.axon_hooks")
    _state = {"hook": _hook_factory()}
    mod.set_axon_ntff_profile_hook = lambda h: _state.__setitem__("hook", h)
    mod.get_axon_ntff_profile_hook = lambda: _state["hook"]
    sys.modules["antenv.axon_hooks"] = mod
    antenv.axon_hooks = mod


# ---------------------------------------------------------------- bass build
def _build_nc(debug_dump=False):
    import concourse.bass as bass  # noqa: F401
    import concourse.tile as tile
    from concourse import bacc, mybir

    BF16 = mybir.dt.bfloat16
    F32 = mybir.dt.float32
    EXP = mybir.ActivationFunctionType.Exp
    CPY = mybir.ActivationFunctionType.Copy
    LN = mybir.ActivationFunctionType.Ln

    nc = bacc.Bacc(None, target_bir_lowering=False, debug=False)

    xT = nc.dram_tensor("xT", [128, 8, 2048], BF16, kind="ExternalInput")
    wq = nc.dram_tensor("wq", [128, 8, 256], BF16, kind="ExternalInput")
    bq = nc.dram_tensor("bq", [128, 2], F32, kind="ExternalInput")
    wl = nc.dram_tensor("wl", [128, 8, 256], BF16, kind="ExternalInput")
    bl = nc.dram_tensor("bl", [128, 2], F32, kind="ExternalInput")
    wk = nc.dram_tensor("wk", [128, 2, 256], BF16, kind="ExternalInput")
    wv = nc.dram_tensor("wv", [128, 2, 256], BF16, kind="ExternalInput")
    wo = nc.dram_tensor("wo", [128, 2, 1024], BF16, kind="ExternalInput")
    out = nc.dram_tensor("out", [2048, 1024], F32, kind="ExternalOutput")

    with nc.allow_low_precision("bf16 intermediates by design"), tile.TileContext(nc) as tc:
        with (
            tc.tile_pool(name="wpool", bufs=1) as wpool,
            tc.tile_pool(name="xpool", bufs=1) as xpool,
            tc.tile_pool(name="proj", bufs=1) as proj,
            tc.tile_pool(name="ptp", bufs=36) as ptp,
            tc.tile_pool(name="atp", bufs=4) as atp,
            tc.tile_pool(name="obp", bufs=4) as obp,
            tc.tile_pool(name="rpool", bufs=2) as rpool,
            tc.tile_pool(name="ps", bufs=2, space="PSUM") as ps,
        ):
            # ---------------- constants + inputs
            x_kn = [
                [xpool.tile([128, 512], BF16, name=f"x_{k}_{n}") for n in range(4)]
                for k in range(8)
            ]
            wq_sb = wpool.tile([128, 8, 256], BF16)
            wl_sb = wpool.tile([128, 8, 256], BF16)
            wk_sb = wpool.tile([128, 2, 256], BF16)
            wv_sb = wpool.tile([128, 2, 256], BF16)
            wo_sb = wpool.tile([128, 2, 1024], BF16)
            bq_sb = wpool.tile([128, 2], F32)
            bl_sb = wpool.tile([128, 2], F32)
            ones_k1 = wpool.tile([128, 64], BF16)
            ones_sb = wpool.tile([128, 1], BF16)
            nc.vector.memset(ones_k1[:], 1.0)
            nc.vector.memset(ones_sb[:], 1.0)

            nc.sync.dma_start(out=wq_sb[:], in_=wq[:])
            nc.sync.dma_start(out=bq_sb[:], in_=bq[:])
            nc.sync.dma_start(out=wl_sb[:], in_=wl[:])
            nc.sync.dma_start(out=bl_sb[:], in_=bl[:])
            nc.sync.dma_start(out=wk_sb[:], in_=wk[:])
            for n in range(4):
                for k in range(8):
                    nc.sync.dma_start(
                        out=x_kn[k][n][:],
                        in_=xT[:, k, 512 * n : 512 * n + 512],
                    )
            nc.sync.dma_start(out=wv_sb[:], in_=wv[:])
            nc.sync.dma_start(out=wo_sb[:], in_=wo[:])

            latT_n = [proj.tile([128, 2, 512], BF16, name=f"latT_{i}") for i in range(4)]
            qT_n = [proj.tile([128, 2, 512], BF16, name=f"qT_{i}") for i in range(4)]
            kT_n = [proj.tile([128, 2, 512], BF16, name=f"kT_{i}") for i in range(4)]
            v_sb = proj.tile([128, 16, 256], BF16)

            def misc_ps(name):
                return ps.tile([128, 512], F32, tag="s", name=name, bufs=3)

            def ot_ps(name):
                return ps.tile([128, 512], F32, tag="ot", name=name, bufs=2)

            # HAM warm-up: dummy matmuls while input DMA is in flight
            warm_sb = wpool.tile([128, 512], BF16)
            nc.vector.memset(warm_sb[:], 0.25)
            warm_ps = misc_ps("warm_ps")
            for i in range(40):
                nc.tensor.matmul(
                    warm_ps[:], warm_sb[:, 0:128], warm_sb[:],
                    start=(i == 0), stop=(i == 39),
                )

            # ---------------- projection emitters (interleaved as fillers)
            def emit_lat(n, m):
                acc = misc_ps(f"lat_ps_{m}_{n}")
                for k in range(8):
                    nc.tensor.matmul(
                        acc[:],
                        wl_sb[:, k, 128 * m : 128 * m + 128],
                        x_kn[k][n][:],
                        start=(k == 0),
                        stop=(k == 7),
                    )
                nc.vector.tensor_scalar_add(
                    out=latT_n[n][:, m, :], in0=acc[:], scalar1=bl_sb[:, m : m + 1]
                )

            def emit_kt(n):
                for m in range(2):
                    acc = misc_ps(f"kt_ps_{m}_{n}")
                    for k in range(2):
                        nc.tensor.matmul(
                            acc[:],
                            wk_sb[:, k, 128 * m : 128 * m + 128],
                            latT_n[n][:, k, :],
                            start=(k == 0),
                            stop=(k == 1),
                        )
                    nc.vector.tensor_copy(out=kT_n[n][:, m, :], in_=acc[:])

            def emit_v(ts):
                for t in ts:
                    acc = misc_ps(f"v_ps_{t}")
                    for k in range(2):
                        nc.tensor.matmul(
                            acc[:, 0:256],
                            latT_n[t // 4][:, k, 128 * (t % 4) : 128 * (t % 4) + 128],
                            wv_sb[:, k, :],
                            start=(k == 0),
                            stop=(k == 1),
                        )
                    nc.vector.tensor_copy(out=v_sb[:, t, :], in_=acc[:, 0:256])

            def emit_qt(n, m):
                acc = misc_ps(f"q_ps_{m}_{n}")
                for k in range(8):
                    nc.tensor.matmul(
                        acc[:],
                        wq_sb[:, k, 128 * m : 128 * m + 128],
                        x_kn[k][n][:],
                        start=(k == 0),
                        stop=(k == 7),
                    )
                nc.vector.tensor_scalar_add(
                    out=qT_n[n][:, m, :], in0=acc[:], scalar1=bq_sb[:, m : m + 1]
                )

            # ---------------- attention phase machinery
            # L psum row per (pair, head-in-pair), chosen so each pvl-adjacent
            # matmul pair has disjoint PE col-groups (concurrency):
            #   pair0: PVh0(cols 0:64) | L_h1@96 ; PVh1(64:128) | L_h0@32
            #   pair1: PVh0 | L_h1@64 ; PVh1 | L_h0@0
            L_ROW = {0: (32, 96), 1: (0, 64)}
            PD = {}
            LT = {}

            def emit_phase(ic, p, fillers):
                """QK + exp for pair (ic,p), kchunk-granular; fillers[t] = closures."""
                key = (ic, p)
                PD[key] = {"pt": []}
                qTc = qT_n[ic]
                for t in range(16):
                    kTc = kT_n[t // 4]
                    ksl = slice(128 * (t % 4), 128 * (t % 4) + 128)
                    s = ps.tile([128, 2, 512], F32, tag="s", name=f"s_{ic}_{p}_{t}", bufs=3)
                    nc.tensor.matmul(
                        s[:, 0, :], kTc[0:64, p, ksl], qTc[0:64, p, :],
                        start=True, stop=True,
                    )
                    nc.tensor.matmul(
                        s[:, 1, :], kTc[64:128, p, ksl], qTc[64:128, p, :],
                        start=True, stop=True,
                    )
                    pt = ptp.tile([128, 2, 512], BF16, tag="pt", name=f"pt_{ic}_{p}_{t}")
                    nc.scalar.activation(pt[:], s[:], EXP)
                    PD[key]["pt"].append(pt)
                    for f in fillers.get(t, ()):
                        f()

            def emit_pvl(key, t):
                """One PV+L kchunk: 4 matmuls ordered for col-group concurrency."""
                d = PD[key]
                ic, p = key
                if t == 0:
                    d["ot"] = ot_ps(f"ot_{ic}_{p}")
                    if p == 0:
                        LT[ic] = misc_ps(f"L_{ic}")
                        nc.vector.memset(LT[ic][:], 1.0)
                Lt = LT[ic]
                pt = d["pt"][t]
                r0, r1 = L_ROW[p]
                h0, h1 = 2 * p, 2 * p + 1
                st, sp = (t == 0), (t == 15)
                nc.tensor.matmul(
                    d["ot"][0:64, :], v_sb[:, t, 64 * h0 : 64 * h0 + 64], pt[:, 0, :],
                    start=st, stop=sp, skip_group_check=True,
                )
                nc.tensor.matmul(
                    Lt[r1 : r1 + 1, :], ones_sb[:], pt[:, 1, :],
                    start=st, stop=sp, tile_position=(0, r1), skip_group_check=True,
                )
                nc.tensor.matmul(
                    d["ot"][64:128, :], v_sb[:, t, 64 * h1 : 64 * h1 + 64], pt[:, 1, :],
                    start=st, stop=sp, skip_group_check=True,
                )
                nc.tensor.matmul(
                    Lt[r0 : r0 + 1, :], ones_sb[:], pt[:, 0, :],
                    start=st, stop=sp, tile_position=(0, r0), skip_group_check=True,
                )

            RC = {}

            def emit_norm_recip(ic):
                """DVE recip of the ic's 4 L rows (emit at end of prev phase)."""
                Lt = LT.pop(ic)
                recip = rpool.tile([128, 512], BF16, tag="recip", name=f"recip_{ic}", bufs=2)
                nc.vector.reciprocal(out=recip[:], in_=Lt[:])
                RC[ic] = recip

            def emit_norm(ic):
                """PE broadcast of recip + DVE mult -> at tiles."""
                recip = RC.pop(ic)
                for p in range(2):
                    d = PD[(ic, p)]
                    bc_ps = misc_ps(f"bcp_{ic}_{p}")
                    for j in range(2):
                        row = L_ROW[p][j]
                        nc.tensor.matmul(
                            bc_ps[64 * j : 64 * j + 64, :],
                            ones_k1[row : row + 1, 0:64],
                            recip[row : row + 1, :],
                            start=True, stop=True,
                            tile_position=(row, 64 * j),
                            skip_group_check=True,
                        )
                    bc = rpool.tile([128, 512], F32, tag="bc", name=f"bcs_{ic}_{p}", bufs=2)
                    nc.vector.tensor_copy(out=bc[:], in_=bc_ps[:])
                    at = atp.tile([128, 512], BF16, tag="at", name=f"at_{ic}_{p}", bufs=4)
                    nc.vector.tensor_mul(out=at[0:64, :], in0=d["ot"][0:64, :], in1=bc[0:64, :])
                    nc.vector.tensor_mul(out=at[64:128, :], in0=d["ot"][64:128, :], in1=bc[64:128, :])
                    d["at"] = at

            def emit_wo_chunk(ic, u):
                at0 = PD[(ic, 0)]["at"]
                at1 = PD[(ic, 1)]["at"]
                for n2 in range(2):
                    wo_ps = ot_ps(f"wo_{ic}_{u}_{n2}")
                    for p, atx in ((0, at0), (1, at1)):
                        nc.tensor.matmul(
                            wo_ps[:],
                            atx[:, 128 * u : 128 * u + 128],
                            wo_sb[:, p, 512 * n2 : 512 * n2 + 512],
                            start=(p == 0),
                            stop=(p == 1),
                        )
                    ob = obp.tile([128, 512], F32, tag="ob", name=f"ob_{ic}_{u}_{n2}")
                    nc.vector.tensor_copy(out=ob[:], in_=wo_ps[:])
                    r0 = 512 * ic + 128 * u
                    nc.sync.dma_start(
                        out=out[r0 : r0 + 128, 512 * n2 : 512 * n2 + 512],
                        in_=ob[:],
                    )

            # ---------------- schedule
            emit_qt(0, 0)
            emit_qt(0, 1)
            emit_lat(0, 0)
            emit_lat(0, 1)
            emit_kt(0)

            def pv_fillers(prev_key, extra=None, tail_fns=()):
                """Spread 16 pvl chunks of prev pair over slots t2..t15."""
                f = {t: [] for t in range(16)}
                if extra:
                    for t, fns in extra.items():
                        f[t].extend(fns)
                slots = [2, 2, 3, 4, 5, 6, 7, 8, 9, 10, 11, 12, 12, 13, 14, 15]
                for k, t in enumerate(slots):
                    f[t].append(lambda k=k: emit_pvl(prev_key, k))
                f[15].extend(tail_fns)
                return f

            # P0: pair (0,0) — projections as fillers
            emit_phase(0, 0, {
                0: [lambda: emit_lat(1, 0)],
                1: [lambda: emit_lat(1, 1)],
                2: [lambda: emit_kt(1), lambda: emit_v(range(0, 2))],
                3: [lambda: emit_v(range(2, 4))],
                4: [lambda: emit_lat(2, 0)],
                5: [lambda: emit_lat(2, 1)],
                6: [lambda: emit_kt(2), lambda: emit_v(range(4, 6))],
                7: [lambda: emit_v(range(6, 8))],
                8: [lambda: emit_lat(3, 0)],
                9: [lambda: emit_lat(3, 1)],
                10: [lambda: emit_kt(3), lambda: emit_v(range(8, 10))],
                11: [lambda: emit_v(range(10, 12))],
                12: [lambda: emit_v(range(12, 14))],
                13: [lambda: emit_v(range(14, 16))],
                14: [lambda: emit_qt(1, 0)],
                15: [lambda: emit_qt(1, 1)],
            })
            # P1: pair (0,1) — qt(2) + PVL(0,0)
            emit_phase(0, 1, pv_fillers((0, 0), extra={
                0: [lambda: emit_qt(2, 0)],
                1: [lambda: emit_qt(2, 1)],
            }))
            # P2: pair (1,0) — qt(3) + PVL(0,1) + recip(0)
            emit_phase(1, 0, pv_fillers((0, 1), extra={
                0: [lambda: emit_qt(3, 0)],
                1: [lambda: emit_qt(3, 1)],
            }, tail_fns=[lambda: emit_norm_recip(0)]))
            # P3: pair (1,1) — norm(0) + Wo(0) + PVL(1,0)
            emit_phase(1, 1, pv_fillers((1, 0), extra={
                0: [lambda: emit_norm(0)],
                **{4 + u: [lambda u=u: emit_wo_chunk(0, u)] for u in range(4)},
            }))
            # P4: pair (2,0) — PVL(1,1) + recip(1)
            emit_phase(2, 0, pv_fillers((1, 1), tail_fns=[lambda: emit_norm_recip(1)]))
            # P5: pair (2,1) — norm(1) + Wo(1) + PVL(2,0)
            emit_phase(2, 1, pv_fillers((2, 0), extra={
                0: [lambda: emit_norm(1)],
                **{4 + u: [lambda u=u: emit_wo_chunk(1, u)] for u in range(4)},
            }))
            # P6: pair (3,0) — PVL(2,1) + recip(2)
            emit_phase(3, 0, pv_fillers((2, 1), tail_fns=[lambda: emit_norm_recip(2)]))
            # P7: pair (3,1) — norm(2) + Wo(2) + PVL(3,0)
            emit_phase(3, 1, pv_fillers((3, 0), extra={
                0: [lambda: emit_norm(2)],
                **{4 + u: [lambda u=u: emit_wo_chunk(2, u)] for u in range(4)},
            }))
            # tail
            for k in range(16):
                emit_pvl((3, 1), k)
            emit_norm_recip(3)
            emit_norm(3)
            for u in range(4):
                emit_wo_chunk(3, u)

    nc.compile()
    return nc


def _get_nc():
    if "nc" not in _STATE:
        _STATE["nc"] = _build_nc()
    return _STATE["nc"]


# ---------------------------------------------------------------- host side
def _pack_k(a, kchunks):
    """[K, N] f32/bf16 -> [128, kchunks, N] bf16 (K = 128*kchunks)."""
    K, N = a.shape
    return np.ascontiguousarray(
        np.asarray(a, np.float32).reshape(kchunks, 128, N).transpose(1, 0, 2)
    ).astype(NPBF16)


def kernel(x, Wq, bq, Wl, bl, Wk, bk, Wv, bv, Wo, bo):
    x = np.asarray(x, np.float32)
    Wq = np.asarray(Wq, np.float32)
    bq = np.asarray(bq, np.float32)
    Wl = np.asarray(Wl, np.float32)
    bl = np.asarray(bl, np.float32)
    Wk = np.asarray(Wk, np.float32)
    Wv = np.asarray(Wv, np.float32)
    bv = np.asarray(bv, np.float32)
    Wo = np.asarray(Wo, np.float32)
    bo = np.asarray(bo, np.float32)

    from concourse.bass_utils import run_bass_kernel_spmd

    trace = os.environ.get("KERNEL_TRACE", "0") == "1"
    if trace:
        _install_ntff_shim()

    wl_p = _pack_k(Wl, 8)
    bl_p = np.ascontiguousarray(bl.reshape(2, 128).T).astype(np.float32)
    in_maps = []
    for c in range(8):
        b, g = divmod(c, 4)
        sl = slice(256 * g, 256 * g + 256)
        in_maps.append(
            {
                "xT": _pack_k(x[b].T, 8),
                "wq": _pack_k(Wq[:, sl] * SCALE, 8),
                "bq": np.ascontiguousarray((bq[sl] * SCALE).reshape(2, 128).T).astype(np.float32),
                "wl": wl_p,
                "bl": bl_p,
                "wk": _pack_k(Wk[:, sl], 2),
                "wv": _pack_k(Wv[:, sl], 2),
                "wo": _pack_k(Wo[sl, :], 2),
            }
        )

    nc = _get_nc()
    res = run_bass_kernel_spmd(nc, in_maps, core_ids=list(range(8)), trace=trace)
    if trace and res.exec_time_ns is not None:
        print(f"HW exec time: {res.exec_time_ns} ns")
        _STATE["exec_time_ns"] = res.exec_time_ns

    parts = [np.asarray(res.results[c]["out"], np.float32) for c in range(8)]
    const = (bv @ Wo + bo).astype(np.float32)
    out = np.empty((2, 2048, 1024), np.float32)
    for b in range(2):
        out[b] = parts[4 * b] + parts[4 * b + 1] + parts[4 * b + 2] + parts[4 * b + 3] + const
    return out


# revision 17
# speedup vs baseline: 1.3000x; 1.2698x over previous
"""MLA attention kernel for TRN2, SPMD over 8 NeuronCores.

Sharding: core c = 4*b + g  (b = batch 0..1, g = head-group 0..3, 4 heads each).
Each core computes, for its batch b and head-group g:
    qT = (Wq_g*scale)^T x^T + bq_g*scale        [256, 2048]   (bf16)
    latT = Wl^T x^T + bl                        [256, 2048]
    kT = Wk_g^T latT                            [256, 2048]   (bk dropped: softmax shift-invariant)
    v  = latT^T Wv_g                            [2048, 256]   (bv folded into host const)
    per head h: sT = kT_h^T qT_h ; pT = exp(sT) (no max-subtraction; scores ~ N(0,1))
                oT = v_h^T pT ; L = 1^T pT ; aT = oT / L
    partial = A Wo_g                            [2048, 1024]  (f32)
Host sums the 4 partials per batch and adds (bv @ Wo + bo).

Emission is kchunk-granular: per k-chunk t the two QK matmuls (row-groups 0/64)
are adjacent for PE row-tile concurrency; PV/L matmuls are ordered
[PVh0, L_h1, PVh1, L_h0] with disjoint col-groups for the same reason.
"""
import contextlib
import ctypes
import os
import sys
import types

if "/opt/trn_rl_repo" not in sys.path:
    sys.path.insert(0, "/opt/trn_rl_repo")

import numpy as np
import ml_dtypes

NPBF16 = ml_dtypes.bfloat16
SCALE = 64 ** -0.5
_STATE = {}


# ---------------------------------------------------------------- ntff shim
def _install_ntff_shim():
    """Provide antenv.axon_hooks so run_bass_kernel_spmd(trace=True) works."""
    if "antenv.axon_hooks" in sys.modules:
        return
    try:
        import antenv
    except ImportError:
        return

    so_path = "/opt/axon/libaxon_pjrt.so"

    def _hook_factory():
        try:
            lib = ctypes.CDLL(so_path)
        except OSError:
            return None
        if not hasattr(lib, "axon_start_nrt_profile"):
            return None
        lib.axon_start_nrt_profile.argtypes = [ctypes.POINTER(ctypes.c_int64), ctypes.c_size_t]
        lib.axon_start_nrt_profile.restype = ctypes.c_int64
        lib.axon_stop_nrt_profile.argtypes = [ctypes.c_char_p]
        lib.axon_stop_nrt_profile.restype = ctypes.c_int64

        @contextlib.contextmanager
        def _hook(output_dir, device_ids):
            import jax

            jax.devices()
            if device_ids:
                ids = (ctypes.c_int64 * len(device_ids))(*device_ids)
                rc = lib.axon_start_nrt_profile(ids, len(device_ids))
            else:
                rc = lib.axon_start_nrt_profile(None, 0)
            if rc != 0:
                raise RuntimeError(f"axon_start_nrt_profile rc={rc}")
            try:
                yield
            finally:
                n = lib.axon_stop_nrt_profile(str(output_dir).encode())
                print(f"profile: {n} file(s) written to {output_dir}", file=sys.stderr)

        return _hook

    import antenv

    mod = types.ModuleType("antenv.axon_hooks")
    _state = {"hook": _hook_factory()}
    mod.set_axon_ntff_profile_hook = lambda h: _state.__setitem__("hook", h)
    mod.get_axon_ntff_profile_hook = lambda: _state["hook"]
    sys.modules["antenv.axon_hooks"] = mod
    antenv.axon_hooks = mod


# ---------------------------------------------------------------- bass build
def _build_nc(debug_dump=False):
    import concourse.bass as bass  # noqa: F401
    import concourse.tile as tile
    from concourse import bacc, mybir

    BF16 = mybir.dt.bfloat16
    F32 = mybir.dt.float32
    EXP = mybir.ActivationFunctionType.Exp
    CPY = mybir.ActivationFunctionType.Copy
    LN = mybir.ActivationFunctionType.Ln

    nc = bacc.Bacc(None, target_bir_lowering=False, debug=False)

    xT = nc.dram_tensor("xT", [128, 8, 2048], BF16, kind="ExternalInput")
    wq = nc.dram_tensor("wq", [128, 8, 256], BF16, kind="ExternalInput")
    bq = nc.dram_tensor("bq", [128, 2], F32, kind="ExternalInput")
    wl = nc.dram_tensor("wl", [128, 8, 256], BF16, kind="ExternalInput")
    bl = nc.dram_tensor("bl", [128, 2], F32, kind="ExternalInput")
    wk = nc.dram_tensor("wk", [128, 2, 256], BF16, kind="ExternalInput")
    wv = nc.dram_tensor("wv", [128, 2, 256], BF16, kind="ExternalInput")
    wo = nc.dram_tensor("wo", [128, 2, 1024], BF16, kind="ExternalInput")
    out = nc.dram_tensor("out", [2048, 1024], F32, kind="ExternalOutput")

    with nc.allow_low_precision("bf16 intermediates by design"), tile.TileContext(nc) as tc:
        with (
            tc.tile_pool(name="wpool", bufs=1) as wpool,
            tc.tile_pool(name="xpool", bufs=1) as xpool,
            tc.tile_pool(name="proj", bufs=1) as proj,
            tc.tile_pool(name="ptp", bufs=36) as ptp,
            tc.tile_pool(name="atp", bufs=4) as atp,
            tc.tile_pool(name="obp", bufs=4) as obp,
            tc.tile_pool(name="rpool", bufs=2) as rpool,
            tc.tile_pool(name="ps", bufs=2, space="PSUM") as ps,
        ):
            # ---------------- constants + inputs
            x_kn = [
                [xpool.tile([128, 512], BF16, name=f"x_{k}_{n}") for n in range(4)]
                for k in range(8)
            ]
            wq_sb = wpool.tile([128, 8, 256], BF16)
            wl_sb = wpool.tile([128, 8, 256], BF16)
            wk_sb = wpool.tile([128, 2, 256], BF16)
            wv_sb = wpool.tile([128, 2, 256], BF16)
            wo_sb = wpool.tile([128, 2, 1024], BF16)
            bq_sb = wpool.tile([128, 2], F32)
            bl_sb = wpool.tile([128, 2], F32)
            ones_k1 = wpool.tile([128, 64], BF16)
            ones_sb = wpool.tile([128, 1], BF16)
            nc.vector.memset(ones_k1[:], 1.0)
            nc.vector.memset(ones_sb[:], 1.0)

            nc.sync.dma_start(out=wq_sb[:], in_=wq[:])
            nc.sync.dma_start(out=bq_sb[:], in_=bq[:])
            nc.sync.dma_start(out=wl_sb[:], in_=wl[:])
            nc.sync.dma_start(out=bl_sb[:], in_=bl[:])
            nc.sync.dma_start(out=wk_sb[:], in_=wk[:])
            for n in range(4):
                for k in range(8):
                    nc.sync.dma_start(
                        out=x_kn[k][n][:],
                        in_=xT[:, k, 512 * n : 512 * n + 512],
                    )
            nc.sync.dma_start(out=wv_sb[:], in_=wv[:])
            nc.sync.dma_start(out=wo_sb[:], in_=wo[:])

            latT_n = [proj.tile([128, 2, 512], BF16, name=f"latT_{i}") for i in range(4)]
            qT_n = [proj.tile([128, 2, 512], BF16, name=f"qT_{i}") for i in range(4)]
            kT_n = [proj.tile([128, 2, 512], BF16, name=f"kT_{i}") for i in range(4)]
            v_sb = proj.tile([128, 16, 256], BF16)

            def misc_ps(name):
                return ps.tile([128, 512], F32, tag="s", name=name, bufs=3)

            def ot_ps(name):
                return ps.tile([128, 512], F32, tag="ot", name=name, bufs=2)

            # HAM warm-up: dummy matmuls while input DMA is in flight
            warm_sb = wpool.tile([128, 512], BF16)
            nc.vector.memset(warm_sb[:], 0.25)
            warm_ps = misc_ps("warm_ps")
            for i in range(40):
                nc.tensor.matmul(
                    warm_ps[:], warm_sb[:, 0:128], warm_sb[:],
                    start=(i == 0), stop=(i == 39),
                )

            # ---------------- projection emitters (interleaved as fillers)
            def emit_lat(n, m):
                acc = misc_ps(f"lat_ps_{m}_{n}")
                for k in range(8):
                    nc.tensor.matmul(
                        acc[:],
                        wl_sb[:, k, 128 * m : 128 * m + 128],
                        x_kn[k][n][:],
                        start=(k == 0),
                        stop=(k == 7),
                    )
                nc.vector.tensor_scalar_add(
                    out=latT_n[n][:, m, :], in0=acc[:], scalar1=bl_sb[:, m : m + 1]
                )

            def emit_kt(n):
                for m in range(2):
                    acc = misc_ps(f"kt_ps_{m}_{n}")
                    for k in range(2):
                        nc.tensor.matmul(
                            acc[:],
                            wk_sb[:, k, 128 * m : 128 * m + 128],
                            latT_n[n][:, k, :],
                            start=(k == 0),
                            stop=(k == 1),
                        )
                    nc.vector.tensor_copy(out=kT_n[n][:, m, :], in_=acc[:])

            def emit_v(ts):
                for t in ts:
                    acc = misc_ps(f"v_ps_{t}")
                    for k in range(2):
                        nc.tensor.matmul(
                            acc[:, 0:256],
                            latT_n[t // 4][:, k, 128 * (t % 4) : 128 * (t % 4) + 128],
                            wv_sb[:, k, :],
                            start=(k == 0),
                            stop=(k == 1),
                        )
                    nc.vector.tensor_copy(out=v_sb[:, t, :], in_=acc[:, 0:256])

            def emit_qt(n, m):
                acc = misc_ps(f"q_ps_{m}_{n}")
                for k in range(8):
                    nc.tensor.matmul(
                        acc[:],
                        wq_sb[:, k, 128 * m : 128 * m + 128],
                        x_kn[k][n][:],
                        start=(k == 0),
                        stop=(k == 7),
                    )
                nc.vector.tensor_scalar_add(
                    out=qT_n[n][:, m, :], in0=acc[:], scalar1=bq_sb[:, m : m + 1]
                )

            # ---------------- attention phase machinery
            # L psum row per (pair, head-in-pair), chosen so each pvl-adjacent
            # matmul pair has disjoint PE col-groups (concurrency):
            #   pair0: PVh0(cols 0:64) | L_h1@96 ; PVh1(64:128) | L_h0@32
            #   pair1: PVh0 | L_h1@64 ; PVh1 | L_h0@0
            L_ROW = {0: (32, 96), 1: (0, 64)}
            PD = {}
            LT = {}

            def emit_phase(ic, p, fillers):
                """QK + exp for pair (ic,p), kchunk-granular; fillers[t] = closures."""
                key = (ic, p)
                PD[key] = {"pt": []}
                qTc = qT_n[ic]
                for t in range(16):
                    kTc = kT_n[t // 4]
                    ksl = slice(128 * (t % 4), 128 * (t % 4) + 128)
                    s = ps.tile([128, 2, 512], F32, tag="s", name=f"s_{ic}_{p}_{t}", bufs=3)
                    nc.tensor.matmul(
                        s[:, 0, :], kTc[0:64, p, ksl], qTc[0:64, p, :],
                        start=True, stop=True,
                    )
                    nc.tensor.matmul(
                        s[:, 1, :], kTc[64:128, p, ksl], qTc[64:128, p, :],
                        start=True, stop=True,
                    )
                    pt = ptp.tile([128, 2, 512], BF16, tag="pt", name=f"pt_{ic}_{p}_{t}")
                    nc.scalar.activation(pt[:], s[:], EXP)
                    PD[key]["pt"].append(pt)
                    for f in fillers.get(t, ()):
                        f()

            def emit_pv_batch(key, gi):
                """8 PV matmuls (t = 4gi..4gi+3), all (128,64)-geometry."""
                d = PD[key]
                ic, p = key
                if gi == 0:
                    d["ot"] = ot_ps(f"ot_{ic}_{p}")
                    if p == 0:
                        LT[ic] = misc_ps(f"L_{ic}")
                        nc.vector.memset(LT[ic][:], 1.0)
                h0, h1 = 2 * p, 2 * p + 1
                for t in range(4 * gi, 4 * gi + 4):
                    pt = d["pt"][t]
                    st, sp = (t == 0), (t == 15)
                    nc.tensor.matmul(
                        d["ot"][0:64, :], v_sb[:, t, 64 * h0 : 64 * h0 + 64], pt[:, 0, :],
                        start=st, stop=sp, skip_group_check=True,
                    )
                    nc.tensor.matmul(
                        d["ot"][64:128, :], v_sb[:, t, 64 * h1 : 64 * h1 + 64], pt[:, 1, :],
                        start=st, stop=sp, skip_group_check=True,
                    )

            def emit_l_batch(key, gi):
                """8 L matmuls (t = 4gi..4gi+3), all (128,1)-geometry."""
                d = PD[key]
                ic, p = key
                Lt = LT[ic]
                r0, r1 = L_ROW[p]
                for t in range(4 * gi, 4 * gi + 4):
                    pt = d["pt"][t]
                    st, sp = (t == 0), (t == 15)
                    nc.tensor.matmul(
                        Lt[r0 : r0 + 1, :], ones_sb[:], pt[:, 0, :],
                        start=st, stop=sp, tile_position=(0, r0), skip_group_check=True,
                    )
                    nc.tensor.matmul(
                        Lt[r1 : r1 + 1, :], ones_sb[:], pt[:, 1, :],
                        start=st, stop=sp, tile_position=(0, r1), skip_group_check=True,
                    )

            RC = {}

            def emit_norm_recip(ic):
                """DVE recip of the ic's 4 L rows (emit at end of prev phase)."""
                Lt = LT.pop(ic)
                recip = rpool.tile([128, 512], BF16, tag="recip", name=f"recip_{ic}", bufs=2)
                nc.vector.reciprocal(out=recip[:], in_=Lt[:])
                RC[ic] = recip

            def emit_norm(ic):
                """PE broadcast of recip + DVE mult -> at tiles."""
                recip = RC.pop(ic)
                for p in range(2):
                    d = PD[(ic, p)]
                    bc_ps = misc_ps(f"bcp_{ic}_{p}")
                    for j in range(2):
                        row = L_ROW[p][j]
                        nc.tensor.matmul(
                            bc_ps[64 * j : 64 * j + 64, :],
                            ones_k1[row : row + 1, 0:64],
                            recip[row : row + 1, :],
                            start=True, stop=True,
                            tile_position=(row, 64 * j),
                            skip_group_check=True,
                        )
                    bc = rpool.tile([128, 512], F32, tag="bc", name=f"bcs_{ic}_{p}", bufs=2)
                    nc.vector.tensor_copy(out=bc[:], in_=bc_ps[:])
                    at = atp.tile([128, 512], BF16, tag="at", name=f"at_{ic}_{p}", bufs=4)
                    nc.vector.tensor_mul(out=at[0:64, :], in0=d["ot"][0:64, :], in1=bc[0:64, :])
                    nc.vector.tensor_mul(out=at[64:128, :], in0=d["ot"][64:128, :], in1=bc[64:128, :])
                    d["at"] = at

            def emit_wo_chunk(ic, u):
                at0 = PD[(ic, 0)]["at"]
                at1 = PD[(ic, 1)]["at"]
                for n2 in range(2):
                    wo_ps = ot_ps(f"wo_{ic}_{u}_{n2}")
                    for p, atx in ((0, at0), (1, at1)):
                        nc.tensor.matmul(
                            wo_ps[:],
                            atx[:, 128 * u : 128 * u + 128],
                            wo_sb[:, p, 512 * n2 : 512 * n2 + 512],
                            start=(p == 0),
                            stop=(p == 1),
                        )
                    ob = obp.tile([128, 512], F32, tag="ob", name=f"ob_{ic}_{u}_{n2}")
                    nc.vector.tensor_copy(out=ob[:], in_=wo_ps[:])
                    r0 = 512 * ic + 128 * u
                    nc.sync.dma_start(
                        out=out[r0 : r0 + 128, 512 * n2 : 512 * n2 + 512],
                        in_=ob[:],
                    )

            # ---------------- schedule
            emit_qt(0, 0)
            emit_qt(0, 1)
            emit_lat(0, 0)
            emit_lat(0, 1)
            emit_kt(0)

            def pv_fillers(prev_key, extra=None, tail_fns=()):
                """PV batches at slots 2/4/6/8, L batches at 10/12/14/15."""
                f = {t: [] for t in range(16)}
                if extra:
                    for t, fns in extra.items():
                        f[t].extend(fns)
                for gi, t in enumerate((2, 4, 6, 8)):
                    f[t].append(lambda gi=gi: emit_pv_batch(prev_key, gi))
                for gi, t in enumerate((10, 12, 14, 15)):
                    f[t].append(lambda gi=gi: emit_l_batch(prev_key, gi))
                f[15].extend(tail_fns)
                return f

            # P0: pair (0,0) — projections as fillers
            emit_phase(0, 0, {
                0: [lambda: emit_lat(1, 0)],
                1: [lambda: emit_lat(1, 1)],
                2: [lambda: emit_kt(1), lambda: emit_v(range(0, 2))],
                3: [lambda: emit_v(range(2, 4))],
                4: [lambda: emit_lat(2, 0)],
                5: [lambda: emit_lat(2, 1)],
                6: [lambda: emit_kt(2), lambda: emit_v(range(4, 6))],
                7: [lambda: emit_v(range(6, 8))],
                8: [lambda: emit_lat(3, 0)],
                9: [lambda: emit_lat(3, 1)],
                10: [lambda: emit_kt(3), lambda: emit_v(range(8, 10))],
                11: [lambda: emit_v(range(10, 12))],
                12: [lambda: emit_v(range(12, 14))],
                13: [lambda: emit_v(range(14, 16))],
                14: [lambda: emit_qt(1, 0)],
                15: [lambda: emit_qt(1, 1)],
            })
            # P1: pair (0,1) — qt(2) + PVL(0,0)
            emit_phase(0, 1, pv_fillers((0, 0), extra={
                0: [lambda: emit_qt(2, 0)],
                1: [lambda: emit_qt(2, 1)],
            }))
            # P2: pair (1,0) — qt(3) + PVL(0,1) + recip(0)
            emit_phase(1, 0, pv_fillers((0, 1), extra={
                0: [lambda: emit_qt(3, 0)],
                1: [lambda: emit_qt(3, 1)],
            }, tail_fns=[lambda: emit_norm_recip(0)]))
            # P3: pair (1,1) — norm(0) + Wo(0) + PVL(1,0)
            emit_phase(1, 1, pv_fillers((1, 0), extra={
                0: [lambda: emit_norm(0)],
                **{3 + 2 * u: [lambda u=u: emit_wo_chunk(0, u)] for u in range(4)},
            }))
            # P4: pair (2,0) — PVL(1,1) + recip(1)
            emit_phase(2, 0, pv_fillers((1, 1), tail_fns=[lambda: emit_norm_recip(1)]))
            # P5: pair (2,1) — norm(1) + Wo(1) + PVL(2,0)
            emit_phase(2, 1, pv_fillers((2, 0), extra={
                0: [lambda: emit_norm(1)],
                **{3 + 2 * u: [lambda u=u: emit_wo_chunk(1, u)] for u in range(4)},
            }))
            # P6: pair (3,0) — PVL(2,1) + recip(2)
            emit_phase(3, 0, pv_fillers((2, 1), tail_fns=[lambda: emit_norm_recip(2)]))
            # P7: pair (3,1) — norm(2) + Wo(2) + PVL(3,0)
            emit_phase(3, 1, pv_fillers((3, 0), extra={
                0: [lambda: emit_norm(2)],
                **{3 + 2 * u: [lambda u=u: emit_wo_chunk(2, u)] for u in range(4)},
            }))
            # tail
            for gi in range(4):
                emit_pv_batch((3, 1), gi)
            for gi in range(4):
                emit_l_batch((3, 1), gi)
            emit_norm_recip(3)
            emit_norm(3)
            for u in range(4):
                emit_wo_chunk(3, u)

    nc.compile()
    return nc


def _get_nc():
    if "nc" not in _STATE:
        _STATE["nc"] = _build_nc()
    return _STATE["nc"]


# ---------------------------------------------------------------- host side
def _pack_k(a, kchunks):
    """[K, N] f32/bf16 -> [128, kchunks, N] bf16 (K = 128*kchunks)."""
    K, N = a.shape
    return np.ascontiguousarray(
        np.asarray(a, np.float32).reshape(kchunks, 128, N).transpose(1, 0, 2)
    ).astype(NPBF16)


def kernel(x, Wq, bq, Wl, bl, Wk, bk, Wv, bv, Wo, bo):
    x = np.asarray(x, np.float32)
    Wq = np.asarray(Wq, np.float32)
    bq = np.asarray(bq, np.float32)
    Wl = np.asarray(Wl, np.float32)
    bl = np.asarray(bl, np.float32)
    Wk = np.asarray(Wk, np.float32)
    Wv = np.asarray(Wv, np.float32)
    bv = np.asarray(bv, np.float32)
    Wo = np.asarray(Wo, np.float32)
    bo = np.asarray(bo, np.float32)

    from concourse.bass_utils import run_bass_kernel_spmd

    trace = os.environ.get("KERNEL_TRACE", "0") == "1"
    if trace:
        _install_ntff_shim()

    wl_p = _pack_k(Wl, 8)
    bl_p = np.ascontiguousarray(bl.reshape(2, 128).T).astype(np.float32)
    in_maps = []
    for c in range(8):
        b, g = divmod(c, 4)
        sl = slice(256 * g, 256 * g + 256)
        in_maps.append(
            {
                "xT": _pack_k(x[b].T, 8),
                "wq": _pack_k(Wq[:, sl] * SCALE, 8),
                "bq": np.ascontiguousarray((bq[sl] * SCALE).reshape(2, 128).T).astype(np.float32),
                "wl": wl_p,
                "bl": bl_p,
                "wk": _pack_k(Wk[:, sl], 2),
                "wv": _pack_k(Wv[:, sl], 2),
                "wo": _pack_k(Wo[sl, :], 2),
            }
        )

    nc = _get_nc()
    res = run_bass_kernel_spmd(nc, in_maps, core_ids=list(range(8)), trace=trace)
    if trace and res.exec_time_ns is not None:
        print(f"HW exec time: {res.exec_time_ns} ns")
        _STATE["exec_time_ns"] = res.exec_time_ns

    parts = [np.asarray(res.results[c]["out"], np.float32) for c in range(8)]
    const = (bv @ Wo + bo).astype(np.float32)
    out = np.empty((2, 2048, 1024), np.float32)
    for b in range(2):
        out[b] = parts[4 * b] + parts[4 * b + 1] + parts[4 * b + 2] + parts[4 * b + 3] + const
    return out
